# revision 5
# baseline (speedup 1.0000x reference)
"""Trainium2 Bass kernel for nn_NodeBlock (GNN message passing).

Pipeline: segment_sum of edge features onto destination nodes, concat with
node features, 3-layer MLP, LayerNorm.

Sharding: the 800 destination-node blocks (128 nodes each) are assigned to
(core, position) pairs, balanced by per-block overflow-edge count so all 8
cores run an identical program (SPMD) with minimal padding. Each core owns
100 blocks; blocks are processed in groups of 4 (512 nodes) so the MLP
matmuls run at N=512.

Aggregation (per 128-node block): edges are fp16 (hi only; ~5e-4 rel err,
far inside the 2e-2 gate). The first T_ID edges of every node are
"identity-packed" on the host: edge #t of node j sits at row j of tile t, so
the scatter-add is matmul(lhsT=edge_tile, rhs=identity) with a static rhs —
no per-tile one-hot build. Overflow edges (rank >= T_ID) are densely packed
and use the classic one-hot trick: oh = (iota == col_local) built by one DVE
tensor_scalar per tile (fp16 4x mode), then matmul(lhsT=edge_tile, rhs=oh).
All aggregation matmuls accumulate feature-major aggrT[f, j] in one PSUM
bank per 4-block group.

MLP is fp16 (f32 PSUM): h1 = relu(W0a^T natT + W0b^T aggrT + b0), h2 =
relu(W1^T h1 + b1). Mean-centering of LayerNorm is folded into W2 on the
host (W2cg = W2 @ (I - 1/128) @ diag(ln_g)), so h3g = W2cg^T h2 + b2cg is
already centered and gamma-scaled. Variance comes from a ones^T matmul over
Square(h3g); rstd = reciprocal_approx_fast(sqrt(v/128+eps)) broadcast to all
partitions via a K=1 matmul; out = h3g * rstd + beta. Output is written
feature-major fp16 and transposed/permuted back on the host.

The LN-tail matmuls (variance sum + rstd broadcast) of group g are emitted
after group g+1's aggregation matmuls so the in-order PE queue never stalls
on the ACT/DVE tail.
"""

import sys

sys.path.insert(0, "/opt/trn_rl_repo")

import numpy as np

N_CORES = 8
NUM_NODES = 100000
D = 128            # node/edge feature dim
P = 128            # partitions
BLK = 128          # nodes per block
BLOCKS_PER_CORE = 100
TOTAL_BLOCKS = 800
PAD_NODES = TOTAL_BLOCKS * BLK      # 102400
GSIZE = 4                            # blocks per group (MLP batch = 512)
GROUPS = BLOCKS_PER_CORE // GSIZE    # 25
EPS = 1e-5

_nc_cache = {}
last_run_info = {}

TUNE = {"T": 10, "ebufs": 2, "ohbufs": 6, "sbufs": 3, "agbufs": 2,
        "mlpbufs": 3, "pvbufs": 1, "prbufs": 2, "only": None}


def derive_schedule(col):
    """Static schedule from the destination-node array only.

    Returns dict with:
      T         identity-packed tiles per block
      ovf       tuple[100] of overflow one-hot tiles per position
      blocks    [100, 8] global block id at (position, core)
    """
    T = TUNE["T"]
    col = np.asarray(col, np.int64)
    deg = np.bincount(col, minlength=PAD_NODES)
    ovf_node = np.maximum(deg - T, 0)
    ovf_blk = ovf_node.reshape(TOTAL_BLOCKS, BLK).sum(axis=1)
    order = np.argsort(-ovf_blk, kind="stable")
    blocks = order.reshape(BLOCKS_PER_CORE, N_CORES)
    grp_max = ovf_blk[blocks].max(axis=1)
    ovf = tuple(int(x) for x in np.ceil(grp_max / BLK).astype(np.int64))
    return {"T": T, "ovf": ovf, "blocks": blocks}


def _build_nc(sched, loop_iters=None):
    import contextlib
    import concourse.bacc as bacc
    import concourse.tile as tile
    import concourse.mybir as mybir

    dt = mybir.dt
    f32 = dt.float32
    f16 = dt.float16
    T = sched["T"]
    ovf = list(sched["ovf"])
    tiles_per_pos = [T + o for o in ovf]
    tot_tiles = sum(tiles_per_pos)
    tot_ovf = sum(ovf)
    grp_tiles = [sum(tiles_per_pos[g * GSIZE:(g + 1) * GSIZE])
                 for g in range(GROUPS)]
    gmax = max(grp_tiles)

    nc = bacc.Bacc("TRN2", target_bir_lowering=False, debug=False,
                   name="nodeblock")

    edges = nc.dram_tensor("edges", [P, tot_tiles * BLK], f16,
                           kind="ExternalInput")
    colf32 = nc.dram_tensor("colf32", [P, max(tot_ovf, 1)], f32,
                            kind="ExternalInput")
    natT = nc.dram_tensor("natT", [P, BLOCKS_PER_CORE * BLK], f16,
                          kind="ExternalInput")
    iota = nc.dram_tensor("iota", [P, BLK], f16, kind="ExternalInput")
    ident = nc.dram_tensor("ident", [P, BLK], f16, kind="ExternalInput")
    onesk = nc.dram_tensor("onesk", [P, 1], f16, kind="ExternalInput")
    onesb = nc.dram_tensor("onesb", [1, P], f16, kind="ExternalInput")
    w_in = {}
    for nm in ["w0a", "w0b", "w1", "w2cg"]:
        w_in[nm] = nc.dram_tensor(nm, [128, 128], f16, kind="ExternalInput")
    for nm in ["b0", "b1", "b2cg", "bet"]:
        w_in[nm] = nc.dram_tensor(nm, [128, 1], f32, kind="ExternalInput")
    out = nc.dram_tensor("out", [GROUPS, P, GSIZE * BLK], f16,
                         kind="ExternalOutput")

    with tile.TileContext(nc) as tc:
        with (
            tc.tile_pool(name="const", bufs=1) as cpool,
            tc.tile_pool(name="edge", bufs=TUNE["ebufs"]) as epool,
            tc.tile_pool(name="oh", bufs=TUNE["ohbufs"]) as ohpool,
            tc.tile_pool(name="small", bufs=TUNE["sbufs"]) as spool,
            tc.tile_pool(name="psag", bufs=TUNE["agbufs"],
                         space="PSUM") as psag,
            tc.tile_pool(name="psmlp", bufs=TUNE["mlpbufs"],
                         space="PSUM") as psmlp,
            tc.tile_pool(name="pspv", bufs=TUNE["pvbufs"],
                         space="PSUM") as pspv,
            tc.tile_pool(name="psprb", bufs=TUNE["prbufs"],
                         space="PSUM") as psprb,
        ):
            colf32_s = cpool.tile([P, max(tot_ovf, 1)], f32, tag="colf32",
                                  name="colf32")
            nc.scalar.dma_start(out=colf32_s[:], in_=colf32[:])
            natT_s = cpool.tile([P, BLOCKS_PER_CORE * BLK], f16, tag="natT",
                                name="natT")
            nc.scalar.dma_start(out=natT_s[:], in_=natT[:])
            iota_s = cpool.tile([P, BLK], f16, tag="iota", name="iota")
            nc.scalar.dma_start(out=iota_s[:], in_=iota[:])
            ident_s = cpool.tile([P, BLK], f16, tag="ident", name="ident")
            nc.scalar.dma_start(out=ident_s[:], in_=ident[:])
            onesk_s = cpool.tile([P, 1], f16, tag="onesk", name="onesk")
            nc.scalar.dma_start(out=onesk_s[:], in_=onesk[:])
            onesb_s = cpool.tile([1, P], f16, tag="onesb", name="onesb")
            nc.scalar.dma_start(out=onesb_s[:], in_=onesb[:])
            consts = {}
            for nm, t in w_in.items():
                consts[nm] = cpool.tile(list(t.shape), t.dtype, tag=nm,
                                        name=nm)
                nc.scalar.dma_start(out=consts[nm][:], in_=t[:])
            epst = cpool.tile([1, 1], dt.float32, tag="epst", name="epst")
            nc.vector.memset(epst[:], EPS)
            zbias = cpool.tile([P, 1], dt.float32, tag="zbias", name="zbias")
            nc.vector.memset(zbias[:], 0.0)
            consts["eps"] = epst
            consts["zero"] = zbias

            loop_cm = (tc.For_i(0, loop_iters, 1) if loop_iters
                       else contextlib.nullcontext())
            with loop_cm:
                _emit_body(nc, sched, grp_tiles, gmax, epool, ohpool, spool,
                           psag, psmlp, pspv, psprb, colf32_s, natT_s,
                           iota_s, ident_s, onesk_s, onesb_s, consts,
                           edges, out, mybir)
    nc.finalize()
    return nc


def _emit_body(nc, sched, grp_tiles, gmax, epool, ohpool, spool, psag,
               psmlp, pspv, psprb, colf32_s, natT_s, iota_s, ident_s,
               onesk_s, onesb_s, consts, edges, out, mybir):
    dt = mybir.dt
    f32 = dt.float32
    f16 = dt.float16
    Alu = mybir.AluOpType
    Act = mybir.ActivationFunctionType
    T = sched["T"]
    ovf = list(sched["ovf"])
    only = TUNE["only"]

    e_off = 0       # column offset into edges dram
    c_off = 0       # column offset into colf32
    pend = None     # deferred LN tail of the previous group

    def emit_tail(p):
        """LN tail of a finished group: variance matmul, rstd, broadcast,
        normalize, output DMA."""
        g, sq, h3g, odma = p
        pv = pspv.tile([1, 512], f32, tag="pv", name="pv")
        nc.tensor.matmul(out=pv[:], lhsT=onesk_s[:], rhs=sq[:],
                         start=True, stop=True)
        std32 = spool.tile([1, 512], f32, tag="std32", name="std32")
        nc.scalar.activation(std32[:], pv[:], Act.Sqrt,
                             bias=consts["eps"][:], scale=1.0 / 128.0)
        rstd32 = spool.tile([1, 512], f32, tag="rstd32", name="rstd32")
        nc.vector.reciprocal_approx_fast(out=rstd32[:], in_=std32[:])
        rstd16 = spool.tile([1, 512], f16, tag="rstd16", name="rstd16")
        nc.vector.tensor_copy(rstd16[:], rstd32[:])
        prb = psprb.tile([P, 512], f32, tag="prb", name="prb")
        nc.tensor.matmul(out=prb[:], lhsT=onesb_s[:], rhs=rstd16[:],
                         start=True, stop=True)
        xn = spool.tile([P, 512], f16, tag="xn", name="xn")
        nc.vector.tensor_tensor(out=xn[:], in0=h3g[:], in1=prb[:],
                                op=Alu.mult)
        yo = spool.tile([P, 512], f16, tag="yo", name="yo")
        nc.vector.tensor_scalar(out=yo[:], in0=xn[:],
                                scalar1=consts["bet"][:], scalar2=None,
                                op0=Alu.add)
        odma.dma_start(out=out[g], in_=yo[:])

    for g in range(GROUPS):
        edma = nc.sync if g % 2 == 0 else nc.scalar
        odma = nc.scalar if g % 2 == 0 else nc.sync
        gcols = grp_tiles[g] * BLK
        eblk = epool.tile([P, gmax * BLK], f16, tag="eblk", name="eblk")
        if only != "nodma":
            edma.dma_start(out=eblk[:, :gcols],
                           in_=edges[:, e_off:e_off + gcols])
        e_off += gcols
        if only == "dma":
            c_off += sum(ovf[g * GSIZE:(g + 1) * GSIZE])
            continue

        pag = psag.tile([P, 512], f32, tag="ag", name="ag")
        off = 0
        for q in range(GSIZE):
            pos = g * GSIZE + q
            K = ovf[pos]
            dst = pag[:, q * BLK:(q + 1) * BLK]
            for t in range(T):
                nc.tensor.matmul(out=dst,
                                 lhsT=eblk[:, (off + t) * BLK:
                                           (off + t + 1) * BLK],
                                 rhs=ident_s[:],
                                 start=(t == 0), stop=(t == T - 1 and K == 0))
            for k in range(K):
                oh = ohpool.tile([P, BLK], f16, tag="oh", name="oh")
                nc.vector.tensor_scalar(
                    out=oh[:], in0=iota_s[:],
                    scalar1=colf32_s[:, c_off + k:c_off + k + 1],
                    scalar2=None, op0=Alu.is_equal)
                nc.tensor.matmul(out=dst,
                                 lhsT=eblk[:, (off + T + k) * BLK:
                                           (off + T + k + 1) * BLK],
                                 rhs=oh[:],
                                 start=False, stop=(k == K - 1))
            off += T + K
            c_off += K

        if only == "agg":
            aggrT = spool.tile([P, 512], f16, tag="aggrT", name="aggrT")
            nc.scalar.copy(aggrT[:], pag[:])
            continue

        aggrT = spool.tile([P, 512], f16, tag="aggrT", name="aggrT")
        nc.scalar.copy(aggrT[:], pag[:])

        # MLP (fp16, f32 PSUM)
        ph1 = psmlp.tile([P, 512], f32, tag="mlp", name="mlp")
        nc.tensor.matmul(out=ph1[:], lhsT=consts["w0a"][:],
                         rhs=natT_s[:, g * 512:(g + 1) * 512],
                         start=True, stop=False)
        nc.tensor.matmul(out=ph1[:], lhsT=consts["w0b"][:], rhs=aggrT[:],
                         start=False, stop=True)
        h1 = spool.tile([P, 512], f16, tag="h1", name="h1")
        nc.scalar.activation(h1[:], ph1[:], Act.Relu, bias=consts["b0"][:])
        ph2 = psmlp.tile([P, 512], f32, tag="mlp", name="mlp")
        nc.tensor.matmul(out=ph2[:], lhsT=consts["w1"][:], rhs=h1[:],
                         start=True, stop=True)
        h2 = spool.tile([P, 512], f16, tag="h2", name="h2")
        nc.scalar.activation(h2[:], ph2[:], Act.Relu, bias=consts["b1"][:])
        ph3 = psmlp.tile([P, 512], f32, tag="mlp", name="mlp")
        nc.tensor.matmul(out=ph3[:], lhsT=consts["w2cg"][:], rhs=h2[:],
                         start=True, stop=True)
        h3g = spool.tile([P, 512], f16, tag="h3g", name="h3g")
        nc.scalar.activation(h3g[:], ph3[:], Act.Identity,
                             bias=consts["b2cg"][:])
        sq = spool.tile([P, 512], f16, tag="sq", name="sq")
        nc.scalar.activation(sq[:], h3g[:], Act.Square,
                             bias=consts["zero"][:])

        if pend is not None:
            emit_tail(pend)
        pend = (g, sq, h3g, odma)

    if pend is not None:
        emit_tail(pend)


def _prepare_shards(node_attr, edge_attr, col, sched):
    """Build per-core edge/col/node buffers per the schedule."""
    T = sched["T"]
    ovf = np.asarray(sched["ovf"], np.int64)
    blocks = sched["blocks"]            # [100 pos, 8 core] global block id
    tiles_per_pos = T + ovf
    tile_base = np.zeros(BLOCKS_PER_CORE + 1, np.int64)
    tile_base[1:] = np.cumsum(tiles_per_pos)
    n_tiles = int(tile_base[-1])
    ovf_base = np.zeros(BLOCKS_PER_CORE + 1, np.int64)
    ovf_base[1:] = np.cumsum(ovf)
    n_ovf_tiles = int(ovf_base[-1])

    # per-block -> (core, position)
    pos_of = np.empty(TOTAL_BLOCKS, np.int64)
    core_of = np.empty(TOTAL_BLOCKS, np.int64)
    for p in range(BLOCKS_PER_CORE):
        for c in range(N_CORES):
            b = blocks[p, c]
            pos_of[b] = p
            core_of[b] = c

    E = col.shape[0]
    # rank of each edge within its destination node (order irrelevant)
    order = np.argsort(col, kind="stable")
    col_s = col[order]
    starts = np.zeros(PAD_NODES + 1, np.int64)
    starts[1:] = np.cumsum(np.bincount(col, minlength=PAD_NODES))
    rank_s = np.arange(E, dtype=np.int64) - starts[col_s]

    blk_s = col_s >> 7
    loc_s = col_s & 127
    pos_s = pos_of[blk_s]
    core_s = core_of[blk_s]

    id_mask = rank_s < T
    # overflow dense rank within each block (edges already sorted by node)
    ov_mask = ~id_mask
    ov_cnt = np.cumsum(ov_mask)
    blk_start_idx = np.searchsorted(blk_s, np.arange(TOTAL_BLOCKS))
    ov_before_blk = np.zeros(TOTAL_BLOCKS, np.int64)
    ov_before_blk[1:] = np.where(blk_start_idx[1:] > 0,
                                 ov_cnt[blk_start_idx[1:] - 1], 0)
    ov_rank = ov_cnt - 1 - ov_before_blk[blk_s]   # valid where ov_mask

    # slot within the core's tile array
    tile_idx = np.where(
        id_mask,
        tile_base[pos_s] + rank_s,
        tile_base[pos_s] + T + (ov_rank >> 7))
    row_idx = np.where(id_mask, loc_s, ov_rank & 127)
    flat = tile_idx * BLK + row_idx

    ea16 = np.ascontiguousarray(edge_attr.astype(np.float16))

    edges_by_core = []
    colf_by_core = []
    natT_by_core = []
    natp = np.zeros((PAD_NODES, D), np.float32)
    natp[:NUM_NODES] = node_attr
    natp = natp.reshape(TOTAL_BLOCKS, BLK, D)
    for c in range(N_CORES):
        m = core_s == c
        buf = np.zeros((n_tiles * BLK, D), np.float16)
        buf[flat[m]] = ea16[order[m]]
        earr = np.ascontiguousarray(
            buf.reshape(n_tiles, BLK, D).transpose(1, 0, 2)
            .reshape(P, n_tiles * BLK))
        edges_by_core.append(earr)

        cbuf = np.full((max(n_ovf_tiles, 1) * BLK,), -1.0, np.float32)
        mo = m & ov_mask
        oslot = (ovf_base[pos_s[mo]] + (ov_rank[mo] >> 7)) * BLK \
            + (ov_rank[mo] & 127)
        cbuf[oslot] = loc_s[mo].astype(np.float32)
        colf_by_core.append(np.ascontiguousarray(
            cbuf.reshape(max(n_ovf_tiles, 1), BLK).T))

        nat_c = natp[blocks[:, c]]                 # [100, 128, D]
        natT_by_core.append(np.ascontiguousarray(
            nat_c.reshape(BLOCKS_PER_CORE * BLK, D).T.astype(np.float16)))
    return edges_by_core, colf_by_core, natT_by_core


def unshard(outs, sched):
    """outs: list of 8 arrays [GROUPS, P, 512] -> full [NUM_NODES, D] f32."""
    blocks = sched["blocks"]
    full = np.zeros((TOTAL_BLOCKS, BLK, D), np.float32)
    for c in range(N_CORES):
        o = np.asarray(outs[c], np.float32).reshape(GROUPS, P, GSIZE, BLK)
        o = o.transpose(0, 2, 3, 1).reshape(BLOCKS_PER_CORE, BLK, D)
        full[blocks[:, c]] = o
    return full.reshape(PAD_NODES, D)[:NUM_NODES]


def kernel(node_attr, edge_attr, edge_index, W0, b0, W1, b1, W2, b2,
           ln_g, ln_b):
    from concourse import bass_utils

    node_attr = np.ascontiguousarray(np.asarray(node_attr, dtype=np.float32))
    edge_attr = np.ascontiguousarray(np.asarray(edge_attr, dtype=np.float32))
    col = np.asarray(edge_index)[1].astype(np.int64)
    W0 = np.asarray(W0, dtype=np.float32)
    W1 = np.asarray(W1, dtype=np.float32)
    W2 = np.asarray(W2, dtype=np.float32)
    ln_g = np.asarray(ln_g, dtype=np.float32)
    ln_b = np.asarray(ln_b, dtype=np.float32)
    b0v = np.asarray(b0, dtype=np.float32).reshape(128, 1).copy()
    b1v = np.asarray(b1, dtype=np.float32).reshape(128, 1).copy()
    b2 = np.asarray(b2, dtype=np.float32)

    # fold LayerNorm mean-centering and gamma into W2/b2
    C = np.eye(D, dtype=np.float64) - 1.0 / D
    w2cg = ((W2.astype(np.float64) @ C) * ln_g[None, :]).astype(np.float16)
    b2cg = ((b2.astype(np.float64) @ C) * ln_g).astype(np.float32) \
        .reshape(128, 1)

    sched = derive_schedule(col)
    edges_by_core, colf_by_core, natT_by_core = _prepare_shards(
        node_attr, edge_attr, col, sched)

    key = (sched["T"], sched["ovf"])
    if key not in _nc_cache:
        _nc_cache[key] = _build_nc(sched)
    nc = _nc_cache[key]

    iota_t = np.ascontiguousarray(
        np.broadcast_to(np.arange(BLK, dtype=np.float16), (P, BLK)))
    ident_t = np.eye(BLK, dtype=np.float16)
    shared = {
        "iota": iota_t, "ident": ident_t,
        "onesk": np.ones((P, 1), np.float16),
        "onesb": np.ones((1, P), np.float16),
        "w0a": np.ascontiguousarray(W0[:128].astype(np.float16)),
        "w0b": np.ascontiguousarray(W0[128:].astype(np.float16)),
        "w1": np.ascontiguousarray(W1.astype(np.float16)),
        "w2cg": np.ascontiguousarray(w2cg),
        "b0": b0v, "b1": b1v, "b2cg": b2cg,
        "bet": ln_b.reshape(128, 1).copy(),
    }
    in_maps = []
    for c in range(N_CORES):
        m = {"edges": edges_by_core[c], "colf32": colf_by_core[c],
             "natT": natT_by_core[c]}
        m.update(shared)
        in_maps.append(m)

    res = bass_utils.run_bass_kernel_spmd(nc, in_maps,
                                          core_ids=list(range(N_CORES)))
    last_run_info["results"] = res
    last_run_info["nc"] = nc
    last_run_info["in_maps"] = in_maps
    last_run_info["kb"] = sched

    outs = [res.results[c]["out"] for c in range(N_CORES)]
    return unshard(outs, sched).astype(np.float32)
